# revision 3
# baseline (speedup 1.0000x reference)
"""CLIP contrastive loss (nn_ClipLoss) on 8 Trainium2 NeuronCores.

Strategy (row-sharded data parallel):
  - Each core k gets its row shard of image/text embeddings (bf16) plus a
    replicated host-transposed bf16 text matrix txtT [D, N] for the matmul
    moving operand.
  - Per-row stats (squared norms, diag dot products) are computed in natural
    layout with tensor_tensor_reduce (free-dim reduction, no partition
    reductions needed).
  - Normalization is folded in as scaling: the image-side 1/||a_i|| (and the
    1/T temperature) rides the ACT exp `scale` per-partition operand; the
    text-side 1/||b_j|| is applied to the streamed txtT tiles after a tiny
    4KB AllGather of per-shard norm scales.
  - logits block per core: [1024, 8192] = aT.T @ bT in bf16, accumulated in
    PSUM fp32, 512-col matmuls, exp on ACT with fused row-sum (accum_out).
  - Column sums accumulate on DVE (bf16), reduced across partitions with a
    ones-matmul, summed across cores with a 32KB ReduceScatter (each core
    gets exactly its own column shard -> no per-core addressing).
  - Each core emits one fp32 partial; host sums 8 partials.
"""

import math

import numpy as np
import ml_dtypes

N_FULL = 8192
D_FULL = 1024
W = 8
P = 128
NSLICE = 512
TEMP = 0.07
LN_INV_T = math.log(1.0 / TEMP)

_CACHE: dict = {}


def build_bass(n_global: int = N_FULL, d: int = D_FULL):
    """Build the SPMD bass program (identical on all cores)."""
    from contextlib import ExitStack

    import concourse.mybir as mybir
    import concourse.tile as tile
    from concourse import bacc

    f32 = mybir.dt.float32
    bf16 = mybir.dt.bfloat16
    AF = mybir.ActivationFunctionType
    OP = mybir.AluOpType
    X = mybir.AxisListType.X

    m_loc = n_global // W          # rows per core
    dc_n = d // P                  # contraction chunks
    mc_n = m_loc // P              # row chunks per core
    gw = m_loc                     # column-group width (= shard width)
    n2_n = gw // NSLICE            # 512-wide psum slices per group
    assert gw % NSLICE == 0 and n2_n <= 2

    nc = bacc.Bacc("TRN2", target_bir_lowering=False, num_devices=W)
    img = nc.dram_tensor("img", [m_loc, d], bf16, kind="ExternalInput")
    txt = nc.dram_tensor("txt", [m_loc, d], bf16, kind="ExternalInput")
    txtT = nc.dram_tensor("txtT", [d, n_global], bf16, kind="ExternalInput")
    out_d = nc.dram_tensor("partial", [1, 1], f32, kind="ExternalOutput")
    rg = [list(range(W))]

    with tile.TileContext(nc) as tc, ExitStack() as ctx:
        sb = ctx.enter_context(tc.tile_pool(name="sb", bufs=1))
        ps = ctx.enter_context(tc.tile_pool(name="ps", bufs=1, space="PSUM"))
        dram = ctx.enter_context(tc.tile_pool(name="dram", bufs=1, space="DRAM"))

        # constants
        ones_bf = sb.tile([P, 1], bf16, name="ones_bf")
        nc.gpsimd.memset(ones_bf[:], 1.0)
        ones_f32 = sb.tile([P, 1], f32, name="ones_f32")
        nc.gpsimd.memset(ones_f32[:], 1.0)
        ones_row_f32 = sb.tile([1, P], f32, name="ones_row_f32")
        nc.gpsimd.memset(ones_row_f32[:], 1.0)

        # collective DRAM buffers
        cc_rb_in = dram.tile([1, m_loc], f32, name="cc_rb_in")
        cc_rb_out = dram.tile([1, n_global], f32, name="cc_rb_out", addr_space="Shared")
        cc_rs_in = dram.tile([1, n_global], f32, name="cc_rs_in")
        cc_rs_out = dram.tile([1, m_loc], f32, name="cc_rs_out")

        # ---------------- prologue: per-row stats in natural layout ----------
        norms2_a = sb.tile([P, mc_n], f32, name="norms2_a")
        norms2_b = sb.tile([P, mc_n], f32, name="norms2_b")
        d_nat = sb.tile([P, mc_n], f32, name="d_nat")

        for m in range(mc_n):
            a_nat = sb.tile([P, d], bf16, name="a_nat", tag="a_nat", bufs=2)
            nc.sync.dma_start(a_nat[:], img[m * P:(m + 1) * P, :])
            b_nat = sb.tile([P, d], bf16, name="b_nat", tag="b_nat", bufs=2)
            nc.sync.dma_start(b_nat[:], txt[m * P:(m + 1) * P, :])
            sqa = sb.tile([P, d], bf16, name="sqa", tag="sqa", bufs=2)
            nc.scalar.square(sqa[:], a_nat[:])
            nc.vector.reduce_sum(norms2_a[:, m:m + 1], sqa[:], axis=X)
            sqb = sb.tile([P, d], bf16, name="sqb", tag="sqb", bufs=2)
            nc.scalar.square(sqb[:], b_nat[:])
            nc.vector.reduce_sum(norms2_b[:, m:m + 1], sqb[:], axis=X)
            prod = sb.tile([P, d], bf16, name="prod", tag="prod", bufs=2)
            nc.vector.tensor_mul(prod[:], a_nat[:], b_nat[:])
            nc.vector.reduce_sum(d_nat[:, m:m + 1], prod[:], axis=X)

        # scales: ra = (1/T) / ||a_i||  (per-partition, fp32, feeds ACT exp)
        ln_invt = sb.tile([P, 1], f32, name="ln_invt")
        nc.gpsimd.memset(ln_invt[:], LN_INV_T)
        ln_a = sb.tile([P, mc_n], f32, name="ln_a")
        nc.scalar.activation(ln_a[:], norms2_a[:], AF.Ln)
        ra_act = sb.tile([P, mc_n], f32, name="ra_act")
        nc.scalar.activation(ra_act[:], ln_a[:], AF.Exp, scale=-0.5, bias=ln_invt[:])

        ln_b = sb.tile([P, mc_n], f32, name="ln_b")
        nc.scalar.activation(ln_b[:], norms2_b[:], AF.Ln)
        rb_loc = sb.tile([P, mc_n], f32, name="rb_loc")
        nc.scalar.activation(rb_loc[:], ln_b[:], AF.Exp, scale=-0.5)

        # diag terms: (a_i . b_i) * ra_i * rb_i   (1/T included via ra_act)
        dterm = sb.tile([P, mc_n], f32, name="dterm")
        nc.vector.tensor_mul(dterm[:], d_nat[:], ra_act[:])
        nc.vector.tensor_mul(dterm[:], dterm[:], rb_loc[:])

        # AllGather text-side scales: 4KB fp32
        # local element order must be j_local = m*128 + p
        nc.sync.dma_start(
            cc_rb_in[0:1, :].rearrange("o (m p) -> p (o m)", p=P), rb_loc[:]
        )
        nc.gpsimd.collective_compute(
            "AllGather",
            OP.bypass,
            replica_groups=rg,
            ins=[cc_rb_in[:].opt()],
            outs=[cc_rb_out[:].opt()],
        )

        # aT: transposed image shard via xbar DMA transpose (bf16)
        aT = []
        for dc in range(dc_n):
            t = sb.tile([P, m_loc], bf16, name=f"aT{dc}")
            nc.sync.dma_start_transpose(t[:], img[:, dc * P:(dc + 1) * P])
            aT.append(t)

        # row-sum accumulator: slot (g*mc_n + m) <- sum_j exp over group g
        row_acc = sb.tile([P, W * mc_n], f32, name="row_acc")

        # ---------------- main loop over column groups -----------------------
        for g in range(W):
            # per-group text scale row, broadcast to 128 partitions via outer
            # product with a ones column (K=1 matmul)
            rbB = sb.tile([P, gw], bf16, name="rbB", tag="rbB", bufs=2)
            for n2 in range(n2_n):
                rb_chunk = sb.tile([1, NSLICE], f32, name="rb_chunk",
                                   tag="rb_chunk", bufs=2)
                nc.sync.dma_start(
                    rb_chunk[:],
                    cc_rb_out[0:1, g * gw + n2 * NSLICE: g * gw + (n2 + 1) * NSLICE],
                )
                bc_ps = ps.tile([P, NSLICE], f32, name="bc_ps", tag="bc_ps", bufs=2)
                nc.tensor.matmul(bc_ps[:], ones_row_f32[:], rb_chunk[:],
                                 start=True, stop=True)
                nc.scalar.copy(rbB[:, n2 * NSLICE:(n2 + 1) * NSLICE], bc_ps[:])

            # stream this group's txtT tiles and scale them
            bTg = []
            for dc in range(dc_n):
                t = sb.tile([P, gw], bf16, name="bT", tag=f"bT{dc}", bufs=2)
                nc.sync.dma_start(
                    t[:], txtT[dc * P:(dc + 1) * P, g * gw:(g + 1) * gw]
                )
                nc.vector.tensor_mul(t[:], t[:], rbB[:])
                bTg.append(t)

            col_acc = sb.tile([P, gw], bf16, name="col_acc", tag="col_acc", bufs=2)
            for m in range(mc_n):
                mm_ps = ps.tile([P, gw], f32, name="mm_ps", tag="mm", bufs=2)
                for dc in range(dc_n):
                    lhsT = aT[dc][:, m * P:(m + 1) * P]
                    for n2 in range(n2_n):
                        nc.tensor.matmul(
                            mm_ps[:, n2 * NSLICE:(n2 + 1) * NSLICE],
                            lhsT,
                            bTg[dc][:, n2 * NSLICE:(n2 + 1) * NSLICE],
                            start=(dc == 0),
                            stop=(dc == dc_n - 1),
                        )
                exp_t = sb.tile([P, gw], bf16, name="exp_t", tag="exp_t", bufs=4)
                slot = g * mc_n + m
                nc.scalar.activation(
                    exp_t[:],
                    mm_ps[:],
                    AF.Exp,
                    scale=ra_act[:, m:m + 1],
                    accum_out=row_acc[:, slot:slot + 1],
                )
                if m == 0:
                    nc.vector.tensor_copy(col_acc[:], exp_t[:])
                else:
                    nc.vector.tensor_add(col_acc[:], col_acc[:], exp_t[:])

            # partition-reduce the group's column sums; stage for ReduceScatter
            cs_row = sb.tile([1, gw], f32, name="cs_row", tag="cs_row", bufs=2)
            for n2 in range(n2_n):
                cs_ps = ps.tile([1, NSLICE], f32, name="cs_ps", tag="cs_ps", bufs=2)
                nc.tensor.matmul(
                    cs_ps[:], ones_bf[:],
                    col_acc[:, n2 * NSLICE:(n2 + 1) * NSLICE],
                    start=True, stop=True,
                )
                nc.scalar.copy(cs_row[0:1, n2 * NSLICE:(n2 + 1) * NSLICE], cs_ps[:])
            nc.sync.dma_start(cc_rs_in[0:1, g * gw:(g + 1) * gw], cs_row[:])

        # ---------------- epilogue -------------------------------------------
        nc.gpsimd.collective_compute(
            "ReduceScatter",
            OP.add,
            replica_groups=rg,
            ins=[cc_rs_in[:].opt()],
            outs=[cc_rs_out[:].opt()],
        )

        # my column shard's summed exp: [P, mc_n] (element order irrelevant)
        scol = sb.tile([P, mc_n], f32, name="scol")
        nc.sync.dma_start(
            scol[:], cc_rs_out[0:1, :].rearrange("o (p f) -> p (o f)", p=P)
        )
        lsc = sb.tile([P, mc_n], f32, name="lsc")
        nc.scalar.activation(lsc[:], scol[:], AF.Ln)

        # total row sums: sum slots over g for each m
        srow = sb.tile([P, mc_n], f32, name="srow")
        nc.vector.tensor_reduce(
            srow[:],
            row_acc[:].rearrange("p (g m) -> p m g", g=W),
            axis=X,
            op=OP.add,
        )
        lsr = sb.tile([P, mc_n], f32, name="lsr")
        nc.scalar.activation(lsr[:], srow[:], AF.Ln)

        # per-partition combine: F = 0.5*(sum lsr + sum lsc) - sum dterm
        s1 = sb.tile([P, 1], f32, name="s1")
        nc.vector.tensor_reduce(s1[:], lsr[:], axis=X, op=OP.add)
        s2 = sb.tile([P, 1], f32, name="s2")
        nc.vector.tensor_reduce(s2[:], lsc[:], axis=X, op=OP.add)
        s3 = sb.tile([P, 1], f32, name="s3")
        nc.vector.tensor_reduce(s3[:], dterm[:], axis=X, op=OP.add)
        tsum = sb.tile([P, 1], f32, name="tsum")
        nc.vector.tensor_add(tsum[:], s1[:], s2[:])
        fvec = sb.tile([P, 1], f32, name="fvec")
        nc.vector.scalar_tensor_tensor(
            out=fvec[:], in0=tsum[:], scalar=0.5, in1=s3[:],
            op0=OP.mult, op1=OP.subtract,
        )

        # partition sum -> scalar partial (scaled by 1/N)
        loss_ps = ps.tile([1, 1], f32, name="loss_ps", tag="cs_ps", bufs=2)
        nc.tensor.matmul(loss_ps[:], ones_f32[:], fvec[:], start=True, stop=True)
        out_sb = sb.tile([1, 1], f32, name="out_sb")
        nc.scalar.mul(out_sb[:], loss_ps[:], 1.0 / n_global)
        nc.sync.dma_start(out_d[0:1, 0:1], out_sb[:])

    nc.compile()
    return nc


def make_in_maps(image_embeddings: np.ndarray, text_embeddings: np.ndarray):
    n_global, d = image_embeddings.shape
    m_loc = n_global // W
    img_bf = image_embeddings.astype(ml_dtypes.bfloat16)
    txt_bf = text_embeddings.astype(ml_dtypes.bfloat16)
    txtT = np.ascontiguousarray(txt_bf.T)
    return [
        {
            "img": img_bf[k * m_loc:(k + 1) * m_loc],
            "txt": txt_bf[k * m_loc:(k + 1) * m_loc],
            "txtT": txtT,
        }
        for k in range(W)
    ]


def kernel(image_embeddings: np.ndarray, text_embeddings: np.ndarray) -> np.ndarray:
    from concourse.bass_utils import run_bass_kernel_spmd

    n_global, d = image_embeddings.shape
    key = (n_global, d)
    if key not in _CACHE:
        _CACHE[key] = build_bass(n_global, d)
    nc = _CACHE[key]

    in_maps = make_in_maps(
        np.asarray(image_embeddings, np.float32),
        np.asarray(text_embeddings, np.float32),
    )
    res = run_bass_kernel_spmd(nc, in_maps, core_ids=list(range(W)))
    total = sum(float(r["partial"][0, 0]) for r in res.results)
    return np.float32(total)


# revision 25
# speedup vs baseline: 25.8002x; 25.8002x over previous
"""CLIP contrastive loss (nn_ClipLoss) on 8 Trainium2 NeuronCores.

Strategy (row-sharded data parallel):
  - Each core k gets its row shard of image/text embeddings (bf16) plus a
    replicated host-transposed bf16 text matrix txtT [D, N] for the matmul
    moving operand.
  - Per-row stats (squared norms, diag dot products) are computed in natural
    layout with tensor_tensor_reduce (free-dim reduction, no partition
    reductions needed).
  - Normalization is folded in as scaling: the image-side 1/||a_i|| (and the
    1/T temperature) rides the ACT exp `scale` per-partition operand; the
    text-side 1/||b_j|| is applied to the streamed txtT tiles after a tiny
    4KB AllGather of per-shard norm scales.
  - logits block per core: [1024, 8192] = aT.T @ bT in bf16, accumulated in
    PSUM fp32, 512-col matmuls, exp on ACT with fused row-sum (accum_out).
  - Column sums accumulate on DVE (bf16), reduced across partitions with a
    ones-matmul, summed across cores with a 32KB ReduceScatter (each core
    gets exactly its own column shard -> no per-core addressing).
  - Each core emits one fp32 partial; host sums 8 partials.
"""

import math

import numpy as np
import ml_dtypes

N_FULL = 8192
D_FULL = 1024
W = 8
P = 128
NSLICE = 512
TEMP = 0.07
LN_INV_T = math.log(1.0 / TEMP)

_CACHE: dict = {}


def build_bass(n_global: int = N_FULL, d: int = D_FULL, collectives: bool = True):
    """Build the SPMD bass program (identical on all cores).

    collectives=False replaces the two collectives with local DMA stand-ins
    (for single-core TimelineSim cost modeling only — numerically wrong
    across cores, but dependency/traffic equivalent on one core).
    """
    from contextlib import ExitStack

    import concourse.mybir as mybir
    import concourse.tile as tile
    from concourse import bacc

    f32 = mybir.dt.float32
    bf16 = mybir.dt.bfloat16
    AF = mybir.ActivationFunctionType
    OP = mybir.AluOpType
    X = mybir.AxisListType.X

    m_loc = n_global // W          # rows per core
    dc_n = d // P                  # contraction chunks
    mc_n = m_loc // P              # row chunks per core
    gw = m_loc                     # column-group width (= shard width)
    n2_n = gw // NSLICE            # 512-wide psum slices per group
    assert gw % NSLICE == 0 and n2_n <= 2

    import concourse.bacc as bacc_mod

    if not getattr(bacc_mod, "_clip_act_tables_patched", False):
        _orig_tabs = bacc_mod.get_activation_tables

        def _one_set_tables(module_arch):
            tabs = dict(_orig_tabs(module_arch))
            full_name = "natural_log_exp_and_others"
            if full_name in tabs:
                ours = {AF.Ln, AF.Exp, AF.Copy, AF.Identity, AF.Square}
                for name in tabs:
                    if name != full_name:
                        tabs[name] = set(tabs[name]) - ours
            return tabs

        bacc_mod.get_activation_tables = _one_set_tables
        bacc_mod._clip_act_tables_patched = True

    nc = bacc.Bacc("TRN2", target_bir_lowering=False, num_devices=W)
    img = nc.dram_tensor("img", [m_loc, d], bf16, kind="ExternalInput")
    txt = nc.dram_tensor("txt", [m_loc, d], bf16, kind="ExternalInput")
    txtT = nc.dram_tensor("txtT", [d, n_global], bf16, kind="ExternalInput")
    imgT = nc.dram_tensor("imgT", [d, m_loc], bf16, kind="ExternalInput")
    out_d = nc.dram_tensor("partial", [1, 1], f32, kind="ExternalOutput")
    rg = [list(range(W))]

    with tile.TileContext(nc) as tc, ExitStack() as ctx:
        sb = ctx.enter_context(tc.tile_pool(name="sb", bufs=1))
        ps = ctx.enter_context(tc.tile_pool(name="ps", bufs=1, space="PSUM"))
        dram = ctx.enter_context(tc.tile_pool(name="dram", bufs=1, space="DRAM"))

        # constants
        ones_bf = sb.tile([P, 1], bf16, name="ones_bf")
        nc.gpsimd.memset(ones_bf[:], 1.0)
        ones_f32 = sb.tile([P, 1], f32, name="ones_f32")
        nc.gpsimd.memset(ones_f32[:], 1.0)
        ones_row_f32 = sb.tile([1, P], f32, name="ones_row_f32")
        nc.gpsimd.memset(ones_row_f32[:], 1.0)

        # collective DRAM buffers
        cc_rb_in = dram.tile([1, m_loc], f32, name="cc_rb_in")
        cc_rb_out = dram.tile(
            [1, n_global], f32, name="cc_rb_out",
            addr_space="Shared" if collectives else "Local",
        )
        cc_rs_in = dram.tile([1, n_global], f32, name="cc_rs_in")
        cc_rs_out = dram.tile([1, m_loc], f32, name="cc_rs_out")

        # ---------------- prologue ------------------------------------------
        # a_nat loads go first: they gate the longest chain (DVE norms ->
        # ra_act -> first exp). Then the first-matmul tiles (aT + group-0 bT).
        # All big streams ride the SP HWDGE queue in this priority order.
        a_nats = []
        b_nats = []
        aT = []
        bT0 = []
        for dc in range(dc_n):
            if dc < mc_n:
                a_nat = sb.tile([P, d], bf16, name="a_nat", tag="a_nat",
                                bufs=mc_n)
                nc.sync.dma_start(a_nat[:], img[dc * P:(dc + 1) * P, :])
                a_nats.append(a_nat)
            t = sb.tile([P, m_loc], bf16, name=f"aT{dc}")
            nc.sync.dma_start(t[:], imgT[dc * P:(dc + 1) * P, :])
            aT.append(t)
            # group-0 text tiles, raw (their scales are applied to the PSUM)
            t2 = sb.tile([P, gw], bf16, name="bT", tag=f"bT{dc}", bufs=3)
            nc.sync.dma_start(t2[:], txtT[dc * P:(dc + 1) * P, 0:gw])
            bT0.append(t2)
            if dc < mc_n:
                b_nat = sb.tile([P, d], bf16, name="b_nat", tag="b_nat",
                                bufs=mc_n)
                nc.sync.dma_start(b_nat[:], txt[dc * P:(dc + 1) * P, :])
                b_nats.append(b_nat)
        for m in range(len(a_nats), mc_n):
            a_nat = sb.tile([P, d], bf16, name="a_nat", tag="a_nat", bufs=mc_n)
            nc.sync.dma_start(a_nat[:], img[m * P:(m + 1) * P, :])
            a_nats.append(a_nat)
            b_nat = sb.tile([P, d], bf16, name="b_nat", tag="b_nat", bufs=mc_n)
            nc.sync.dma_start(b_nat[:], txt[m * P:(m + 1) * P, :])
            b_nats.append(b_nat)

        # prefetch group 1's text tiles ahead of the later group streams
        bT_pre = {0: bT0}
        if W > 1:
            bT1 = []
            for dc in range(dc_n):
                t = sb.tile([P, gw], bf16, name="bT", tag=f"bT{dc}", bufs=3)
                nc.sync.dma_start(t[:], txtT[dc * P:(dc + 1) * P, gw:2 * gw])
                bT1.append(t)
            bT_pre[1] = bT1

        norms2_a = sb.tile([P, mc_n], f32, name="norms2_a")
        norms2_b = sb.tile([P, mc_n], f32, name="norms2_b")
        d_nat = sb.tile([P, mc_n], f32, name="d_nat")
        ln_invt = sb.tile([P, 1], f32, name="ln_invt")
        nc.gpsimd.memset(ln_invt[:], LN_INV_T)

        # a-side norms on DVE (loads already issued above)
        for m in range(mc_n):
            sqa = sb.tile([P, d], bf16, name="sqa", tag="sqa", bufs=2)
            nc.vector.scalar_tensor_tensor(
                out=sqa[:], in0=a_nats[m][:], scalar=1.0, in1=a_nats[m][:],
                op0=OP.mult, op1=OP.mult, accum_out=norms2_a[:, m:m + 1],
            )

        # ---- group 0 fast path: compute its text norms locally from txtT
        # (no dependency on the AllGather). ACT squares + DVE adds.
        sq_acc = sb.tile([P, gw], bf16, name="sq_acc")
        for dc, t in enumerate(bT0):
            if dc == 0:
                nc.scalar.activation(sq_acc[:], t[:], AF.Square)
            else:
                sq0 = sb.tile([P, gw], bf16, name="sq0", tag="sq0", bufs=2)
                nc.scalar.activation(sq0[:], t[:], AF.Square)
                nc.vector.tensor_add(sq_acc[:], sq_acc[:], sq0[:])
        g0_ln = sb.tile([1, gw], f32, name="g0_ln")
        for n2 in range(n2_n):
            g0_ps = ps.tile([1, NSLICE], f32, name="g0_ps", tag="cs_ps", bufs=2)
            nc.tensor.matmul(
                g0_ps[:], ones_bf[:], sq_acc[:, n2 * NSLICE:(n2 + 1) * NSLICE],
                start=True, stop=True,
            )
            nc.scalar.activation(
                g0_ln[0:1, n2 * NSLICE:(n2 + 1) * NSLICE], g0_ps[:], AF.Ln
            )
        g0_rb = sb.tile([1, gw], f32, name="g0_rb")
        nc.scalar.activation(g0_rb[:], g0_ln[:], AF.Exp, scale=-0.5)
        # fp32 broadcast of group-0 scales; applied to the PSUM logits (not
        # the bf16 tiles) so the group-0 matmuls don't wait on the norms.
        rbB0 = sb.tile([P, gw], f32, name="rbB0")
        for n2 in range(n2_n):
            bc_ps = ps.tile([P, NSLICE], f32, name="bc_ps", tag="bc_ps", bufs=2)
            nc.tensor.matmul(bc_ps[:], ones_row_f32[:],
                             g0_rb[0:1, n2 * NSLICE:(n2 + 1) * NSLICE],
                             start=True, stop=True)
            nc.scalar.copy(rbB0[:, n2 * NSLICE:(n2 + 1) * NSLICE], bc_ps[:])

        # image-side exp scale: ra = (1/T) / ||a_i||
        ln_a = sb.tile([P, mc_n], f32, name="ln_a")
        nc.scalar.activation(ln_a[:], norms2_a[:], AF.Ln)
        ra_act = sb.tile([P, mc_n], f32, name="ra_act")
        nc.scalar.activation(ra_act[:], ln_a[:], AF.Exp, scale=-0.5, bias=ln_invt[:])

        # b-side stats: gate the rb AllGather (needed by group 1 at ~35us)
        for m in range(mc_n):
            sqb = sb.tile([P, d], bf16, name="sqb", tag="sqb", bufs=2)
            nc.vector.scalar_tensor_tensor(
                out=sqb[:], in0=b_nats[m][:], scalar=1.0, in1=b_nats[m][:],
                op0=OP.mult, op1=OP.mult, accum_out=norms2_b[:, m:m + 1],
            )

        ln_b = sb.tile([P, mc_n], f32, name="ln_b")
        nc.scalar.activation(ln_b[:], norms2_b[:], AF.Ln)
        rb_loc = sb.tile([P, mc_n], f32, name="rb_loc")
        nc.scalar.activation(rb_loc[:], ln_b[:], AF.Exp, scale=-0.5)

        # AllGather text-side scales: 4KB fp32, 8 contiguous column-extract
        # DMAs (global element order j_local = m*128 + p)
        for m in range(mc_n):
            nc.gpsimd.dma_start(
                cc_rb_in[0:1, m * P:(m + 1) * P], rb_loc[:, m:m + 1]
            )
        if collectives:
            nc.gpsimd.collective_compute(
                "AllGather",
                OP.bypass,
                replica_groups=rg,
                ins=[cc_rb_in[:].opt()],
                outs=[cc_rb_out[:].opt()],
            )
        else:
            for r in range(W):
                nc.gpsimd.dma_start(
                    cc_rb_out[0:1, r * m_loc:(r + 1) * m_loc], cc_rb_in[:]
                )

        # row-sum accumulator: slot (g*mc_n + m) <- sum_j exp over group g
        row_acc = sb.tile([P, W * mc_n], f32, name="row_acc")

        # ---------------- main loop over column groups -----------------------
        for g in range(W):
            if g == 0:
                bTg = bT_pre[0]
                rbB = rbB0
            else:
                # per-group text scale row from the AllGather, broadcast to 128
                # partitions via outer product with a ones column (K=1 matmul)
                rbB = sb.tile([P, gw], f32, name="rbB", tag="rbB", bufs=2)
                for n2 in range(n2_n):
                    rb_chunk = sb.tile([1, NSLICE], f32, name="rb_chunk",
                                       tag="rb_chunk", bufs=2)
                    nc.gpsimd.dma_start(
                        rb_chunk[:],
                        cc_rb_out[0:1, g * gw + n2 * NSLICE:
                                  g * gw + (n2 + 1) * NSLICE],
                    )
                    bc_ps = ps.tile([P, NSLICE], f32, name="bc_ps",
                                    tag="bc_ps", bufs=2)
                    nc.tensor.matmul(bc_ps[:], ones_row_f32[:], rb_chunk[:],
                                     start=True, stop=True)
                    nc.scalar.copy(rbB[:, n2 * NSLICE:(n2 + 1) * NSLICE], bc_ps[:])

                # stream this group's txtT tiles (raw — scales go on the PSUM)
                if g in bT_pre:
                    bTg = bT_pre[g]
                else:
                    bTg = []
                    for dc in range(dc_n):
                        t = sb.tile([P, gw], bf16, name="bT", tag=f"bT{dc}",
                                    bufs=3)
                        nc.sync.dma_start(
                            t[:], txtT[dc * P:(dc + 1) * P, g * gw:(g + 1) * gw]
                        )
                        bTg.append(t)

            col_acc = sb.tile([P, gw], bf16, name="col_acc", tag="col_acc", bufs=2)
            for m in range(mc_n):
                mm_ps = ps.tile([P, gw], f32, name="mm_ps", tag="mm", bufs=2)
                for dc in range(dc_n):
                    lhsT = aT[dc][:, m * P:(m + 1) * P]
                    for n2 in range(n2_n):
                        nc.tensor.matmul(
                            mm_ps[:, n2 * NSLICE:(n2 + 1) * NSLICE],
                            lhsT,
                            bTg[dc][:, n2 * NSLICE:(n2 + 1) * NSLICE],
                            start=(dc == 0),
                            stop=(dc == dc_n - 1),
                        )
                # apply the text-side scales to the fp32 logits in PSUM
                nc.vector.tensor_mul(mm_ps[:], mm_ps[:], rbB[:])
                exp_t = sb.tile([P, gw], bf16, name="exp_t", tag="exp_t", bufs=4)
                slot = g * mc_n + m
                nc.scalar.activation(
                    exp_t[:],
                    mm_ps[:],
                    AF.Exp,
                    scale=ra_act[:, m:m + 1],
                    accum_out=row_acc[:, slot:slot + 1],
                )
                if m == 0:
                    nc.vector.tensor_copy(col_acc[:], exp_t[:])
                else:
                    nc.vector.tensor_add(col_acc[:], col_acc[:], exp_t[:])

            # partition-reduce the group's column sums; stage for ReduceScatter
            cs_row = sb.tile([1, gw], f32, name="cs_row", tag="cs_row", bufs=2)
            for n2 in range(n2_n):
                cs_ps = ps.tile([1, NSLICE], f32, name="cs_ps", tag="cs_ps", bufs=2)
                nc.tensor.matmul(
                    cs_ps[:], ones_bf[:],
                    col_acc[:, n2 * NSLICE:(n2 + 1) * NSLICE],
                    start=True, stop=True,
                )
                nc.scalar.copy(cs_row[0:1, n2 * NSLICE:(n2 + 1) * NSLICE], cs_ps[:])
            nc.gpsimd.dma_start(cc_rs_in[0:1, g * gw:(g + 1) * gw], cs_row[:])

        # diag terms: one fused DVE op per chunk (mul + free-dim accum);
        # low priority, only the epilogue consumes these.
        for m in range(mc_n):
            prod = sb.tile([P, d], bf16, name="prod", tag="prod", bufs=2)
            nc.vector.scalar_tensor_tensor(
                out=prod[:], in0=a_nats[m][:], scalar=1.0, in1=b_nats[m][:],
                op0=OP.mult, op1=OP.mult, accum_out=d_nat[:, m:m + 1],
            )
        dterm = sb.tile([P, mc_n], f32, name="dterm")
        nc.vector.tensor_mul(dterm[:], d_nat[:], ra_act[:])
        nc.vector.tensor_mul(dterm[:], dterm[:], rb_loc[:])


        # ---------------- epilogue -------------------------------------------
        if collectives:
            nc.gpsimd.collective_compute(
                "ReduceScatter",
                OP.add,
                replica_groups=rg,
                ins=[cc_rs_in[:].opt()],
                outs=[cc_rs_out[:].opt()],
            )
        else:
            nc.gpsimd.dma_start(cc_rs_out[:], cc_rs_in[0:1, 0:m_loc])

        # my column shard's summed exp: [P, mc_n] (element order irrelevant)
        scol = sb.tile([P, mc_n], f32, name="scol")
        nc.gpsimd.dma_start(
            scol[:], cc_rs_out[0:1, :].rearrange("o (p f) -> p (o f)", p=P)
        )
        lsc = sb.tile([P, mc_n], f32, name="lsc")
        nc.scalar.activation(lsc[:], scol[:], AF.Ln)

        # total row sums: sum slots over g for each m
        srow = sb.tile([P, mc_n], f32, name="srow")
        nc.vector.tensor_reduce(
            srow[:],
            row_acc[:].rearrange("p (g m) -> p m g", g=W),
            axis=X,
            op=OP.add,
        )
        lsr = sb.tile([P, mc_n], f32, name="lsr")
        nc.scalar.activation(lsr[:], srow[:], AF.Ln)

        # per-partition combine: F = 0.5*(sum lsr + sum lsc) - sum dterm
        s1 = sb.tile([P, 1], f32, name="s1")
        nc.vector.tensor_reduce(s1[:], lsr[:], axis=X, op=OP.add)
        s2 = sb.tile([P, 1], f32, name="s2")
        nc.vector.tensor_reduce(s2[:], lsc[:], axis=X, op=OP.add)
        s3 = sb.tile([P, 1], f32, name="s3")
        nc.vector.tensor_reduce(s3[:], dterm[:], axis=X, op=OP.add)
        tsum = sb.tile([P, 1], f32, name="tsum")
        nc.vector.tensor_add(tsum[:], s1[:], s2[:])
        fvec = sb.tile([P, 1], f32, name="fvec")
        nc.vector.scalar_tensor_tensor(
            out=fvec[:], in0=tsum[:], scalar=0.5, in1=s3[:],
            op0=OP.mult, op1=OP.subtract,
        )

        # partition sum -> scalar partial (scaled by 1/N)
        loss_ps = ps.tile([1, 1], f32, name="loss_ps", tag="cs_ps", bufs=2)
        nc.tensor.matmul(loss_ps[:], ones_f32[:], fvec[:], start=True, stop=True)
        out_sb = sb.tile([1, 1], f32, name="out_sb")
        nc.scalar.mul(out_sb[:], loss_ps[:], 1.0 / n_global)
        nc.gpsimd.dma_start(out_d[0:1, 0:1], out_sb[:])

    nc.compile()
    return nc


def make_in_maps(image_embeddings: np.ndarray, text_embeddings: np.ndarray):
    n_global, d = image_embeddings.shape
    m_loc = n_global // W
    img_bf = image_embeddings.astype(ml_dtypes.bfloat16)
    txt_bf = text_embeddings.astype(ml_dtypes.bfloat16)
    txtT = np.ascontiguousarray(txt_bf.T)
    return [
        {
            "img": img_bf[k * m_loc:(k + 1) * m_loc],
            "txt": txt_bf[k * m_loc:(k + 1) * m_loc],
            "txtT": txtT,
            "imgT": np.ascontiguousarray(img_bf[k * m_loc:(k + 1) * m_loc].T),
        }
        for k in range(W)
    ]


def kernel(image_embeddings: np.ndarray, text_embeddings: np.ndarray) -> np.ndarray:
    from concourse.bass_utils import run_bass_kernel_spmd

    n_global, d = image_embeddings.shape
    key = (n_global, d)
    if key not in _CACHE:
        _CACHE[key] = build_bass(n_global, d)
    nc = _CACHE[key]

    in_maps = make_in_maps(
        np.asarray(image_embeddings, np.float32),
        np.asarray(text_embeddings, np.float32),
    )
    res = run_bass_kernel_spmd(nc, in_maps, core_ids=list(range(W)))
    total = sum(float(r["partial"][0, 0]) for r in res.results)
    return np.float32(total)
